# revision 1
# baseline (speedup 1.0000x reference)
"""Multi-Head Latent Attention (MLA) Bass kernel for 8 trn2 NeuronCores.

Sharding: core c handles batch b=c//4 and head group hg=c%4 (4 of 16 heads).
The small compression projections (W_dq/W_dkv) are replicated; the per-head
up-projections and W_o are sliced by head. Host transposes x[b] once, and the
whole device pipeline runs in "transposed" layout (feature dims on SBUF
partitions) so no on-device transposes are needed:

  c_qT  [1536, T] = W_dq.T  @ x.T      (lhsT = W_dq, rhs = xT)
  qT_h  [128, T]  = W_uq_h.T @ c_qT
  scoresT [tk, tq] = (lhsT=kT_chunk, rhs=qT)
  exp (ACT, no max-subtraction: logits are O(5)), row sums via ones-matmul,
  out_avT [dh, tq] = (lhsT=v_chunk natural, rhs=expT)
  out    [tq, C]  = (lhsT=out_avT, rhs=W_o rows)   + causal block skipping

All matmuls run in float32r (fp32 stored, fp22 multiply) at 1 cycle/row.
Host sums the 4 partial outputs per batch.
"""

import numpy as np

T = 2048
C = 2048
QC = 1536
KV = 512
NH = 16
DH = 128
R = 64
TB = 512           # T block / q-group width
NTB = T // TB      # 4
SCALE = 1.0 / float(np.sqrt(DH + R))
ROPE_BASE = 10000.0

_CACHE = {}


def _build_nc():
    import concourse.bacc as bacc
    import concourse.mybir as mybir
    import concourse.tile as tile

    F32R = mybir.dt.float32r
    F32 = mybir.dt.float32
    AF = mybir.ActivationFunctionType

    nc = bacc.Bacc("TRN2", target_bir_lowering=False, debug=False)

    xT = nc.dram_tensor("xT", [C, T], F32R, kind="ExternalInput")
    wdq = nc.dram_tensor("wdq", [C, QC], F32R, kind="ExternalInput")
    wdkv = nc.dram_tensor("wdkv", [C, KV], F32R, kind="ExternalInput")
    wuq = nc.dram_tensor("wuq", [QC, 512], F32R, kind="ExternalInput")
    wqr = nc.dram_tensor("wqr", [QC, 256], F32R, kind="ExternalInput")
    wuk = nc.dram_tensor("wuk", [KV, 512], F32R, kind="ExternalInput")
    wuv = nc.dram_tensor("wuv", [KV, 512], F32R, kind="ExternalInput")
    wkr = nc.dram_tensor("wkr", [KV, 256], F32R, kind="ExternalInput")
    wo = nc.dram_tensor("wo", [512, C], F32R, kind="ExternalInput")
    cosd = nc.dram_tensor("cosd", [128, T], F32, kind="ExternalInput")
    sind = nc.dram_tensor("sind", [128, T], F32, kind="ExternalInput")
    maskd = nc.dram_tensor("maskd", [TB, TB], F32, kind="ExternalInput")
    onesd = nc.dram_tensor("onesd", [128, 1], F32R, kind="ExternalInput")
    out = nc.dram_tensor("out", [T, C], F32, kind="ExternalOutput")

    with tile.TileContext(nc) as tc:
        with tc.tile_pool(name="dram", bufs=1, space="DRAM") as dpool:
            qcT = dpool.tile([512, T], F32R, name="qcT")
            qrT = dpool.tile([256, T], F32R, name="qrT")
            kcT = dpool.tile([512, T], F32R, name="kcT")
            krT = dpool.tile([256, T], F32R, name="krT")
            vS = dpool.tile([T, 512], F32R, name="vS")

            # ---------------- Phase P: projections ----------------
            with (
                tc.tile_pool(name="pp_sb", bufs=1) as sp,
                tc.tile_pool(name="pp_ps", bufs=1, space="PSUM") as pp,
            ):
                wuq_sb = sp.tile([128, 12, 512], F32R, name="wuq_sb")
                nc.sync.dma_start(wuq_sb[:], wuq.rearrange("(k p) n -> p k n", p=128))
                wqr_sb = sp.tile([128, 12, 256], F32R, name="wqr_sb")
                nc.sync.dma_start(wqr_sb[:], wqr.rearrange("(k p) n -> p k n", p=128))
                wuk_sb = sp.tile([128, 4, 512], F32R, name="wuk_sb")
                nc.sync.dma_start(wuk_sb[:], wuk.rearrange("(k p) n -> p k n", p=128))
                wuv_sb = sp.tile([128, 4, 512], F32R, name="wuv_sb")
                nc.sync.dma_start(wuv_sb[:], wuv.rearrange("(k p) n -> p k n", p=128))
                wkr_sb = sp.tile([128, 4, 256], F32R, name="wkr_sb")
                nc.sync.dma_start(wkr_sb[:], wkr.rearrange("(k p) n -> p k n", p=128))
                cos_sb = sp.tile([128, T], F32, name="cos_sb")
                nc.sync.dma_start(cos_sb[:], cosd[:])
                sin_sb = sp.tile([128, T], F32, name="sin_sb")
                nc.sync.dma_start(sin_sb[:], sind[:])

                def rope_store(ps_t, dst, p, tc0):
                    # ps_t [128, TB]: rows [64 head 2p | 64 head 2p+1] rope dims
                    t1 = sp.tile([128, TB], F32, name="rp1", tag="rp1", bufs=2)
                    nc.vector.tensor_mul(t1[:], ps_t[:], cos_sb[:, tc0:tc0 + TB])
                    sh = sp.tile([128, TB], F32, name="rp2", tag="rp2", bufs=2)
                    nc.vector.tensor_copy(sh[0:32, :], ps_t[32:64, :])
                    nc.vector.tensor_copy(sh[32:64, :], ps_t[0:32, :])
                    nc.vector.tensor_copy(sh[64:96, :], ps_t[96:128, :])
                    nc.vector.tensor_copy(sh[96:128, :], ps_t[64:96, :])
                    t2 = sp.tile([128, TB], F32, name="rp3", tag="rp3", bufs=2)
                    nc.vector.tensor_mul(t2[:], sh[:], sin_sb[:, tc0:tc0 + TB])
                    ro = sp.tile([128, TB], F32R, name="ro", tag="ro", bufs=2)
                    nc.vector.tensor_add(ro[:], t1[:], t2[:])
                    nc.sync.dma_start(dst[128 * p:128 * (p + 1), tc0:tc0 + TB], ro[:])

                for t in range(NTB):
                    tc0 = TB * t
                    xblk = sp.tile([128, 16, TB], F32R, name="xblk", tag="xblk", bufs=1)
                    for k in range(16):
                        nc.sync.dma_start(
                            xblk[:, k, :], xT[128 * k:128 * (k + 1), tc0:tc0 + TB])
                    # c_qT block [1536, TB]
                    cq_blk = sp.tile([128, 12, TB], F32R, name="cq_blk", tag="cq_blk", bufs=1)
                    for m in range(12):
                        wst = sp.tile([128, 16, 128], F32R, name="wst", tag="wst", bufs=3)
                        nc.sync.dma_start(
                            wst[:], wdq[:, 128 * m:128 * (m + 1)].rearrange(
                                "(k p) n -> p k n", p=128))
                        ps_t = pp.tile([128, TB], F32, name="ps_p", tag="ps_p", bufs=2)
                        for k in range(16):
                            nc.tensor.matmul(ps_t[:], wst[:, k, :], xblk[:, k, :],
                                             start=(k == 0), stop=(k == 15))
                        nc.vector.tensor_copy(cq_blk[:, m, :], ps_t[:])
                    # c_kvT block [512, TB]
                    ckv_blk = sp.tile([128, 4, TB], F32R, name="ckv_blk", tag="ckv_blk", bufs=1)
                    for m in range(4):
                        wst = sp.tile([128, 16, 128], F32R, name="wst", tag="wst", bufs=3)
                        nc.sync.dma_start(
                            wst[:], wdkv[:, 128 * m:128 * (m + 1)].rearrange(
                                "(k p) n -> p k n", p=128))
                        ps_t = pp.tile([128, TB], F32, name="ps_p", tag="ps_p", bufs=2)
                        for k in range(16):
                            nc.tensor.matmul(ps_t[:], wst[:, k, :], xblk[:, k, :],
                                             start=(k == 0), stop=(k == 15))
                        nc.vector.tensor_copy(ckv_blk[:, m, :], ps_t[:])
                    # q content per head
                    for h in range(4):
                        ps_t = pp.tile([128, TB], F32, name="ps_p", tag="ps_p", bufs=2)
                        for k in range(12):
                            nc.tensor.matmul(
                                ps_t[:], wuq_sb[:, k, 128 * h:128 * (h + 1)],
                                cq_blk[:, k, :], start=(k == 0), stop=(k == 11))
                        st = sp.tile([128, TB], F32R, name="stq", tag="stq", bufs=3)
                        nc.vector.tensor_copy(st[:], ps_t[:])
                        nc.sync.dma_start(qcT[128 * h:128 * (h + 1), tc0:tc0 + TB], st[:])
                    # q rope per head-pair
                    for p in range(2):
                        ps_t = pp.tile([128, TB], F32, name="ps_p", tag="ps_p", bufs=2)
                        for k in range(12):
                            nc.tensor.matmul(
                                ps_t[:], wqr_sb[:, k, 128 * p:128 * (p + 1)],
                                cq_blk[:, k, :], start=(k == 0), stop=(k == 11))
                        rope_store(ps_t, qrT, p, tc0)
                    # k content per head
                    for h in range(4):
                        ps_t = pp.tile([128, TB], F32, name="ps_p", tag="ps_p", bufs=2)
                        for k in range(4):
                            nc.tensor.matmul(
                                ps_t[:], wuk_sb[:, k, 128 * h:128 * (h + 1)],
                                ckv_blk[:, k, :], start=(k == 0), stop=(k == 3))
                        st = sp.tile([128, TB], F32R, name="stk", tag="stk", bufs=3)
                        nc.vector.tensor_copy(st[:], ps_t[:])
                        nc.sync.dma_start(kcT[128 * h:128 * (h + 1), tc0:tc0 + TB], st[:])
                    # k rope per head-pair
                    for p in range(2):
                        ps_t = pp.tile([128, TB], F32, name="ps_p", tag="ps_p", bufs=2)
                        for k in range(4):
                            nc.tensor.matmul(
                                ps_t[:], wkr_sb[:, k, 128 * p:128 * (p + 1)],
                                ckv_blk[:, k, :], start=(k == 0), stop=(k == 3))
                        rope_store(ps_t, krT, p, tc0)
                    # v natural [tk, 4*dh]
                    for tkc in range(4):
                        ps_t = pp.tile([128, TB], F32, name="ps_p", tag="ps_p", bufs=2)
                        for k in range(4):
                            nc.tensor.matmul(
                                ps_t[:], ckv_blk[:, k, 128 * tkc:128 * (tkc + 1)],
                                wuv_sb[:, k, :], start=(k == 0), stop=(k == 3))
                        st = sp.tile([128, TB], F32R, name="stv", tag="stv", bufs=3)
                        nc.vector.tensor_copy(st[:], ps_t[:])
                        nc.sync.dma_start(
                            vS[tc0 + 128 * tkc:tc0 + 128 * (tkc + 1), :], st[:])

            # ---------------- Phase A: attention + W_o ----------------
            with (
                tc.tile_pool(name="pa_sb", bufs=1) as sa,
                tc.tile_pool(name="pa_ps", bufs=1, space="PSUM") as pa,
            ):
                wo_sb = sa.tile([128, 4, C], F32R, name="wo_sb")
                nc.sync.dma_start(wo_sb[:], wo.rearrange("(h p) n -> p h n", p=128))
                mask_sb = sa.tile([128, 4, TB], F32, name="mask_sb")
                nc.sync.dma_start(mask_sb[:], maskd.rearrange("(j p) n -> p j n", p=128))
                ones_sb = sa.tile([128, 1], F32R, name="ones_sb")
                nc.sync.dma_start(ones_sb[:], onesd[:])

                for g in range(4):
                    nch = 4 * (g + 1)
                    tq0 = TB * g
                    avn = sa.tile([128, 4 * TB], F32R, name="avn", tag="avn", bufs=2)
                    for h in range(4):
                        pr = 128 * (h // 2) + 64 * (h % 2)
                        qc_t = sa.tile([128, TB], F32R, name="qc_t", tag="qc_t", bufs=2)
                        nc.sync.dma_start(qc_t[:], qcT[128 * h:128 * (h + 1), tq0:tq0 + TB])
                        qr_t = sa.tile([64, TB], F32R, name="qr_t", tag="qr_t", bufs=2)
                        nc.sync.dma_start(qr_t[:], qrT[pr:pr + 64, tq0:tq0 + TB])
                        kc_t = sa.tile([128, T], F32R, name="kc_t", tag="kc_t", bufs=2)
                        nc.sync.dma_start(kc_t[:, 0:128 * nch],
                                          kcT[128 * h:128 * (h + 1), 0:128 * nch])
                        kr_t = sa.tile([64, T], F32R, name="kr_t", tag="kr_t", bufs=2)
                        nc.sync.dma_start(kr_t[:, 0:128 * nch], krT[pr:pr + 64, 0:128 * nch])
                        v_t = sa.tile([128, 16, 128], F32R, name="v_t", tag="v_t", bufs=2)
                        nc.sync.dma_start(
                            v_t[:, 0:nch, :],
                            vS[0:128 * nch, 128 * h:128 * (h + 1)].rearrange(
                                "(c p) d -> p c d", p=128))

                        ps_av = pa.tile([128, TB], F32, name="ps_av", tag="ps_av", bufs=2)
                        ps_sum = pa.tile([1, TB], F32, name="ps_sum", tag="ps_sum", bufs=2)

                        def qk(c):
                            ps_s = pa.tile([128, TB], F32, name="ps_s", tag="ps_s", bufs=2)
                            nc.tensor.matmul(ps_s[:], kc_t[:, 128 * c:128 * (c + 1)],
                                             qc_t[:], start=True, stop=False)
                            nc.tensor.matmul(ps_s[:], kr_t[:, 128 * c:128 * (c + 1)],
                                             qr_t[:], start=False, stop=True)
                            return ps_s

                        cur = qk(0)
                        for c in range(nch):
                            nxt = qk(c + 1) if c + 1 < nch else None
                            ex = sa.tile([128, TB], F32R, name="ex", tag="ex", bufs=3)
                            if c >= 4 * g:
                                er = sa.tile([128, TB], F32, name="er", tag="er", bufs=2)
                                nc.scalar.activation(er[:], cur[:], AF.Exp, scale=SCALE)
                                nc.vector.tensor_mul(ex[:], er[:], mask_sb[:, c - 4 * g, :])
                            else:
                                nc.scalar.activation(ex[:], cur[:], AF.Exp, scale=SCALE)
                            nc.tensor.matmul(ps_av[:], v_t[:, c, :], ex[:],
                                             start=(c == 0), stop=(c == nch - 1))
                            nc.tensor.matmul(ps_sum[:], ones_sb[:], ex[:],
                                             start=(c == 0), stop=(c == nch - 1))
                            cur = nxt
                        recip = sa.tile([1, TB], F32, name="recip", tag="recip", bufs=2)
                        nc.vector.reciprocal(recip[:], ps_sum[:])
                        rb = sa.tile([1, TB], F32, name="rb", tag="rb", bufs=2, space="DRAM")
                        nc.sync.dma_start(rb[:], recip[:])
                        bc = sa.tile([128, TB], F32, name="bc", tag="bc", bufs=2)
                        nc.sync.dma_start(bc[:], rb[0:1, :].to_broadcast([128, TB]))
                        nc.vector.tensor_mul(avn[:, TB * h:TB * (h + 1)], ps_av[:], bc[:])
                    # W_o partial for this q group
                    for tqc in range(4):
                        for n in range(4):
                            ps_o = pa.tile([128, 512], F32, name="ps_o", tag="ps_o", bufs=2)
                            for h in range(4):
                                nc.tensor.matmul(
                                    ps_o[:],
                                    avn[:, TB * h + 128 * tqc:TB * h + 128 * (tqc + 1)],
                                    wo_sb[:, h, 512 * n:512 * (n + 1)],
                                    start=(h == 0), stop=(h == 3))
                            ost = sa.tile([128, 512], F32, name="ost", tag="ost", bufs=3)
                            nc.vector.tensor_copy(ost[:], ps_o[:])
                            nc.sync.dma_start(
                                out[tq0 + 128 * tqc:tq0 + 128 * (tqc + 1),
                                    512 * n:512 * (n + 1)], ost[:])

    nc.compile()
    return nc


def _rope_tables():
    inv = 1.0 / (ROPE_BASE ** (np.arange(0, R, 2, dtype=np.float32) / R))
    freqs = np.arange(T, dtype=np.float32)[:, None] * inv[None, :]       # [T, 32]
    emb = np.concatenate([freqs, freqs], axis=-1)                         # [T, 64]
    cosT = np.ascontiguousarray(np.cos(emb).T.astype(np.float32))         # [64, T]
    sinT = np.ascontiguousarray(np.sin(emb).T.astype(np.float32))
    cosd = np.concatenate([cosT, cosT], axis=0)                           # [128, T]
    sin_sgn = np.concatenate([-sinT[0:32], sinT[32:64]], axis=0)          # [64, T]
    sind = np.concatenate([sin_sgn, sin_sgn], axis=0)
    return cosd, sind


def kernel(**inputs):
    from concourse.bass_utils import run_bass_kernel_spmd

    x = np.asarray(inputs["x"], dtype=np.float32)
    W_dq = np.asarray(inputs["W_dq"], dtype=np.float32)
    W_uq = np.asarray(inputs["W_uq"], dtype=np.float32)
    W_qr = np.asarray(inputs["W_qr"], dtype=np.float32)
    W_dkv = np.asarray(inputs["W_dkv"], dtype=np.float32)
    W_uk = np.asarray(inputs["W_uk"], dtype=np.float32)
    W_uv = np.asarray(inputs["W_uv"], dtype=np.float32)
    W_kr = np.asarray(inputs["W_kr"], dtype=np.float32)
    W_o = np.asarray(inputs["W_o"], dtype=np.float32)

    if "nc" not in _CACHE:
        _CACHE["nc"] = _build_nc()
    nc = _CACHE["nc"]

    cosd, sind = _rope_tables()
    maskv = (np.arange(TB)[:, None] <= np.arange(TB)[None, :]).astype(np.float32)
    onesv = np.ones((128, 1), dtype=np.float32)

    in_maps = []
    for core in range(8):
        b, hg = core // 4, core % 4
        in_maps.append({
            "xT": np.ascontiguousarray(x[b].T),
            "wdq": W_dq,
            "wdkv": W_dkv,
            "wuq": np.ascontiguousarray(W_uq[:, 512 * hg:512 * (hg + 1)]),
            "wqr": np.ascontiguousarray(W_qr[:, 256 * hg:256 * (hg + 1)]),
            "wuk": np.ascontiguousarray(W_uk[:, 512 * hg:512 * (hg + 1)]),
            "wuv": np.ascontiguousarray(W_uv[:, 512 * hg:512 * (hg + 1)]),
            "wkr": np.ascontiguousarray(W_kr[:, 256 * hg:256 * (hg + 1)]),
            "wo": np.ascontiguousarray(W_o[512 * hg:512 * (hg + 1), :]),
            "cosd": cosd,
            "sind": sind,
            "maskd": maskv,
            "onesd": onesv,
        })

    res = run_bass_kernel_spmd(nc, in_maps, core_ids=list(range(8)))
    outs = [r["out"] for r in res.results]
    out0 = outs[0] + outs[1] + outs[2] + outs[3]
    out1 = outs[4] + outs[5] + outs[6] + outs[7]
    return np.stack([out0, out1]).astype(np.float32)


# revision 5
# speedup vs baseline: 3558.2552x; 3558.2552x over previous
"""Multi-Head Latent Attention (MLA) Bass kernel for 8 trn2 NeuronCores.

Sharding: core c handles batch b=c//4 and head group hg=c%4 (4 of 16 heads).
The small compression projections (W_dq/W_dkv) are replicated; the per-head
up-projections and W_o are sliced by head. Host transposes x[b] once, and the
whole device pipeline runs in "transposed" layout (feature dims on SBUF
partitions) so no on-device transposes are needed:

  c_qT  [1536, T] = W_dq.T  @ x.T      (lhsT = W_dq, rhs = xT)
  qT_h  [128, T]  = W_uq_h.T @ c_qT
  scoresT [tk, tq] = (lhsT=kT_chunk, rhs=qT)
  exp (ACT, no max-subtraction: logits are O(5)), row sums via ones-matmul,
  out_avT [dh, tq] = (lhsT=v_chunk natural, rhs=expT)
  out    [tq, C]  = (lhsT=out_avT, rhs=W_o rows)   + causal block skipping

All matmuls run in float32r (fp32 stored, fp22 multiply) at 1 cycle/row.
Host sums the 4 partial outputs per batch.
"""

import numpy as np

T = 2048
C = 2048
QC = 1536
KV = 512
NH = 16
DH = 128
R = 64
TB = 512           # T block / q-group width
NTB = T // TB      # 4
SCALE = 1.0 / float(np.sqrt(DH + R))
ROPE_BASE = 10000.0

_CACHE = {}


def _build_nc(repeat=1):
    import concourse.bacc as bacc
    import concourse.mybir as mybir
    import concourse.tile as tile

    F32R = mybir.dt.float32r
    F32 = mybir.dt.float32
    AF = mybir.ActivationFunctionType

    nc = bacc.Bacc("TRN2", target_bir_lowering=False, debug=False)

    xT = nc.dram_tensor("xT", [C, T], F32R, kind="ExternalInput")
    wdq = nc.dram_tensor("wdq", [C, QC], F32R, kind="ExternalInput")
    wdkv = nc.dram_tensor("wdkv", [C, KV], F32R, kind="ExternalInput")
    wuq = nc.dram_tensor("wuq", [QC, 512], F32R, kind="ExternalInput")
    wqr = nc.dram_tensor("wqr", [QC, 256], F32R, kind="ExternalInput")
    wuk = nc.dram_tensor("wuk", [KV, 512], F32R, kind="ExternalInput")
    wuv = nc.dram_tensor("wuv", [KV, 512], F32R, kind="ExternalInput")
    wkr = nc.dram_tensor("wkr", [KV, 256], F32R, kind="ExternalInput")
    wo = nc.dram_tensor("wo", [512, C], F32R, kind="ExternalInput")
    cosd = nc.dram_tensor("cosd", [128, T], F32, kind="ExternalInput")
    sind = nc.dram_tensor("sind", [128, T], F32, kind="ExternalInput")
    maskd = nc.dram_tensor("maskd", [TB, TB], F32, kind="ExternalInput")
    onesd = nc.dram_tensor("onesd", [128, 1], F32R, kind="ExternalInput")
    out = nc.dram_tensor("out", [T, C], F32, kind="ExternalOutput")

    with tile.TileContext(nc) as tc:
        with tc.tile_pool(name="dram", bufs=1, space="DRAM") as dpool:
            for _rep in range(repeat):
                _emit_body(nc, tc, dpool, mybir,
                           xT, wdq, wdkv, wuq, wqr, wuk, wuv, wkr, wo,
                           cosd, sind, maskd, onesd, out)

    nc.compile()
    return nc


def _emit_body(nc, tc, dpool, mybir,
               xT, wdq, wdkv, wuq, wqr, wuk, wuv, wkr, wo,
               cosd, sind, maskd, onesd, out):
    F32R = mybir.dt.float32r
    F32 = mybir.dt.float32
    AF = mybir.ActivationFunctionType
    import concourse.tile as tile  # noqa: F401

    if True:
        if True:
            qcT = dpool.tile([512, T], F32R, name="qcT", tag="qcT")
            qrT = dpool.tile([256, T], F32R, name="qrT", tag="qrT")
            kcT = dpool.tile([512, T], F32R, name="kcT", tag="kcT")
            krT = dpool.tile([256, T], F32R, name="krT", tag="krT")
            vS = dpool.tile([T, 512], F32R, name="vS", tag="vS")

            # ---------------- Phase P: projections ----------------
            with (
                tc.tile_pool(name="pp_sb", bufs=1) as sp,
                tc.tile_pool(name="pp_ps", bufs=1, space="PSUM") as pp,
            ):
                # Allocated up front, but the DMAs are emitted after block 0's
                # c_q matmul group so they don't delay the first matmul.
                wuq_sb = sp.tile([128, 12, 512], F32R, name="wuq_sb")
                wqr_sb = sp.tile([128, 12, 256], F32R, name="wqr_sb")
                wuk_sb = sp.tile([128, 4, 512], F32R, name="wuk_sb")
                wuv_sb = sp.tile([128, 4, 512], F32R, name="wuv_sb")
                wkr_sb = sp.tile([128, 4, 256], F32R, name="wkr_sb")
                cos_sb = sp.tile([128, T], F32, name="cos_sb")
                sin_sb = sp.tile([128, T], F32, name="sin_sb")

                def load_resident_weights():
                    nc.sync.dma_start(wuq_sb[:], wuq.rearrange("(k p) n -> p k n", p=128))
                    nc.sync.dma_start(wqr_sb[:], wqr.rearrange("(k p) n -> p k n", p=128))
                    nc.sync.dma_start(wuk_sb[:], wuk.rearrange("(k p) n -> p k n", p=128))
                    nc.sync.dma_start(wuv_sb[:], wuv.rearrange("(k p) n -> p k n", p=128))
                    nc.sync.dma_start(wkr_sb[:], wkr.rearrange("(k p) n -> p k n", p=128))
                    nc.sync.dma_start(cos_sb[:], cosd[:])
                    nc.sync.dma_start(sin_sb[:], sind[:])

                def rope_store(ps_t, dst, p, tc0):
                    # ps_t [128, TB]: rows [64 head 2p | 64 head 2p+1] rope dims
                    t1 = sp.tile([128, TB], F32, name="rp1", tag="rp1", bufs=2)
                    nc.vector.tensor_mul(t1[:], ps_t[:], cos_sb[:, tc0:tc0 + TB])
                    sh = sp.tile([128, TB], F32, name="rp2", tag="rp2", bufs=2)
                    nc.vector.tensor_copy(sh[0:32, :], ps_t[32:64, :])
                    nc.vector.tensor_copy(sh[32:64, :], ps_t[0:32, :])
                    nc.vector.tensor_copy(sh[64:96, :], ps_t[96:128, :])
                    nc.vector.tensor_copy(sh[96:128, :], ps_t[64:96, :])
                    t2 = sp.tile([128, TB], F32, name="rp3", tag="rp3", bufs=2)
                    nc.vector.tensor_mul(t2[:], sh[:], sin_sb[:, tc0:tc0 + TB])
                    ro = sp.tile([128, TB], F32R, name="ro", tag="ro", bufs=2)
                    nc.vector.tensor_add(ro[:], t1[:], t2[:])
                    nc.sync.dma_start(dst[128 * p:128 * (p + 1), tc0:tc0 + TB], ro[:])

                for t in range(NTB):
                    tc0 = TB * t
                    xblk = sp.tile([128, 16, TB], F32R, name="xblk", tag="xblk", bufs=1)
                    for k in range(16):
                        nc.sync.dma_start(
                            xblk[:, k, :], xT[128 * k:128 * (k + 1), tc0:tc0 + TB])
                    # c_qT block [1536, TB]
                    cq_blk = sp.tile([128, 12, TB], F32R, name="cq_blk", tag="cq_blk", bufs=1)
                    for m in range(12):
                        wst = sp.tile([128, 16, 128], F32R, name="wst", tag="wst", bufs=3)
                        nc.sync.dma_start(
                            wst[:], wdq[:, 128 * m:128 * (m + 1)].rearrange(
                                "(k p) n -> p k n", p=128))
                        ps_t = pp.tile([128, TB], F32, name="ps_p", tag="ps_p", bufs=2)
                        for k in range(16):
                            nc.tensor.matmul(ps_t[:], wst[:, k, :], xblk[:, k, :],
                                             start=(k == 0), stop=(k == 15))
                        nc.vector.tensor_copy(cq_blk[:, m, :], ps_t[:])
                    # c_kvT block [512, TB]
                    ckv_blk = sp.tile([128, 4, TB], F32R, name="ckv_blk", tag="ckv_blk", bufs=1)
                    for m in range(4):
                        wst = sp.tile([128, 16, 128], F32R, name="wst", tag="wst", bufs=3)
                        nc.sync.dma_start(
                            wst[:], wdkv[:, 128 * m:128 * (m + 1)].rearrange(
                                "(k p) n -> p k n", p=128))
                        ps_t = pp.tile([128, TB], F32, name="ps_p", tag="ps_p", bufs=2)
                        for k in range(16):
                            nc.tensor.matmul(ps_t[:], wst[:, k, :], xblk[:, k, :],
                                             start=(k == 0), stop=(k == 15))
                        nc.vector.tensor_copy(ckv_blk[:, m, :], ps_t[:])
                    # q content per head
                    for h in range(4):
                        ps_t = pp.tile([128, TB], F32, name="ps_p", tag="ps_p", bufs=2)
                        for k in range(12):
                            nc.tensor.matmul(
                                ps_t[:], wuq_sb[:, k, 128 * h:128 * (h + 1)],
                                cq_blk[:, k, :], start=(k == 0), stop=(k == 11))
                        st = sp.tile([128, TB], F32R, name="stq", tag="stq", bufs=3)
                        nc.vector.tensor_copy(st[:], ps_t[:])
                        nc.sync.dma_start(qcT[128 * h:128 * (h + 1), tc0:tc0 + TB], st[:])
                    # q rope per head-pair
                    for p in range(2):
                        ps_t = pp.tile([128, TB], F32, name="ps_p", tag="ps_p", bufs=2)
                        for k in range(12):
                            nc.tensor.matmul(
                                ps_t[:], wqr_sb[:, k, 128 * p:128 * (p + 1)],
                                cq_blk[:, k, :], start=(k == 0), stop=(k == 11))
                        rope_store(ps_t, qrT, p, tc0)
                    # k content per head
                    for h in range(4):
                        ps_t = pp.tile([128, TB], F32, name="ps_p", tag="ps_p", bufs=2)
                        for k in range(4):
                            nc.tensor.matmul(
                                ps_t[:], wuk_sb[:, k, 128 * h:128 * (h + 1)],
                                ckv_blk[:, k, :], start=(k == 0), stop=(k == 3))
                        st = sp.tile([128, TB], F32R, name="stk", tag="stk", bufs=3)
                        nc.vector.tensor_copy(st[:], ps_t[:])
                        nc.sync.dma_start(kcT[128 * h:128 * (h + 1), tc0:tc0 + TB], st[:])
                    # k rope per head-pair
                    for p in range(2):
                        ps_t = pp.tile([128, TB], F32, name="ps_p", tag="ps_p", bufs=2)
                        for k in range(4):
                            nc.tensor.matmul(
                                ps_t[:], wkr_sb[:, k, 128 * p:128 * (p + 1)],
                                ckv_blk[:, k, :], start=(k == 0), stop=(k == 3))
                        rope_store(ps_t, krT, p, tc0)
                    # v natural [tk, 4*dh]
                    for tkc in range(4):
                        ps_t = pp.tile([128, TB], F32, name="ps_p", tag="ps_p", bufs=2)
                        for k in range(4):
                            nc.tensor.matmul(
                                ps_t[:], ckv_blk[:, k, 128 * tkc:128 * (tkc + 1)],
                                wuv_sb[:, k, :], start=(k == 0), stop=(k == 3))
                        st = sp.tile([128, TB], F32R, name="stv", tag="stv", bufs=3)
                        nc.vector.tensor_copy(st[:], ps_t[:])
                        nc.sync.dma_start(
                            vS[tc0 + 128 * tkc:tc0 + 128 * (tkc + 1), :], st[:])

            # ---------------- Phase A: attention + W_o ----------------
            with (
                tc.tile_pool(name="pa_sb", bufs=1) as sa,
                tc.tile_pool(name="pa_ps", bufs=1, space="PSUM") as pa,
            ):
                wo_sb = sa.tile([128, 4, C], F32R, name="wo_sb")
                nc.sync.dma_start(wo_sb[:], wo.rearrange("(h p) n -> p h n", p=128))
                mask_sb = sa.tile([128, 4, TB], F32, name="mask_sb")
                nc.sync.dma_start(mask_sb[:], maskd.rearrange("(j p) n -> p j n", p=128))
                ones_sb = sa.tile([128, 1], F32R, name="ones_sb")
                nc.sync.dma_start(ones_sb[:], onesd[:])

                for g in range(4):
                    nch = 4 * (g + 1)
                    tq0 = TB * g
                    avn = sa.tile([128, 4 * TB], F32R, name="avn", tag="avn", bufs=2)
                    for h in range(4):
                        pr = 128 * (h // 2) + 64 * (h % 2)
                        qc_t = sa.tile([128, TB], F32R, name="qc_t", tag="qc_t", bufs=2)
                        nc.sync.dma_start(qc_t[:], qcT[128 * h:128 * (h + 1), tq0:tq0 + TB])
                        qr_t = sa.tile([64, TB], F32R, name="qr_t", tag="qr_t", bufs=2)
                        nc.sync.dma_start(qr_t[:], qrT[pr:pr + 64, tq0:tq0 + TB])
                        kc_t = sa.tile([128, T], F32R, name="kc_t", tag="kc_t", bufs=2)
                        nc.sync.dma_start(kc_t[:, 0:128 * nch],
                                          kcT[128 * h:128 * (h + 1), 0:128 * nch])
                        kr_t = sa.tile([64, T], F32R, name="kr_t", tag="kr_t", bufs=2)
                        nc.sync.dma_start(kr_t[:, 0:128 * nch], krT[pr:pr + 64, 0:128 * nch])
                        v_t = sa.tile([128, 16, 128], F32R, name="v_t", tag="v_t", bufs=2)
                        nc.sync.dma_start(
                            v_t[:, 0:nch, :],
                            vS[0:128 * nch, 128 * h:128 * (h + 1)].rearrange(
                                "(c p) d -> p c d", p=128))

                        ps_av = pa.tile([128, TB], F32, name="ps_av", tag="ps_av", bufs=2)
                        ps_sum = pa.tile([1, TB], F32, name="ps_sum", tag="ps_sum", bufs=2)

                        def qk(c):
                            ps_s = pa.tile([128, TB], F32, name="ps_s", tag="ps_s", bufs=2)
                            nc.tensor.matmul(ps_s[:], kc_t[:, 128 * c:128 * (c + 1)],
                                             qc_t[:], start=True, stop=False)
                            nc.tensor.matmul(ps_s[:], kr_t[:, 128 * c:128 * (c + 1)],
                                             qr_t[:], start=False, stop=True)
                            return ps_s

                        cur = qk(0)
                        for c in range(nch):
                            nxt = qk(c + 1) if c + 1 < nch else None
                            ex = sa.tile([128, TB], F32R, name="ex", tag="ex", bufs=3)
                            if c >= 4 * g:
                                er = sa.tile([128, TB], F32, name="er", tag="er", bufs=2)
                                nc.scalar.activation(er[:], cur[:], AF.Exp, scale=SCALE)
                                nc.vector.tensor_mul(ex[:], er[:], mask_sb[:, c - 4 * g, :])
                            else:
                                nc.scalar.activation(ex[:], cur[:], AF.Exp, scale=SCALE)
                            nc.tensor.matmul(ps_av[:], v_t[:, c, :], ex[:],
                                             start=(c == 0), stop=(c == nch - 1))
                            nc.tensor.matmul(ps_sum[:], ones_sb[:], ex[:],
                                             start=(c == 0), stop=(c == nch - 1))
                            cur = nxt
                        recip = sa.tile([1, TB], F32, name="recip", tag="recip", bufs=2)
                        nc.vector.reciprocal(recip[:], ps_sum[:])
                        rb = sa.tile([1, TB], F32, name="rb", tag="rb", bufs=2, space="DRAM")
                        nc.sync.dma_start(rb[:], recip[:])
                        bc = sa.tile([128, TB], F32, name="bc", tag="bc", bufs=2)
                        nc.sync.dma_start(bc[:], rb[0:1, :].to_broadcast([128, TB]))
                        nc.vector.tensor_mul(avn[:, TB * h:TB * (h + 1)], ps_av[:], bc[:])
                    # W_o partial for this q group
                    for tqc in range(4):
                        for n in range(4):
                            ps_o = pa.tile([128, 512], F32, name="ps_o", tag="ps_o", bufs=2)
                            for h in range(4):
                                nc.tensor.matmul(
                                    ps_o[:],
                                    avn[:, TB * h + 128 * tqc:TB * h + 128 * (tqc + 1)],
                                    wo_sb[:, h, 512 * n:512 * (n + 1)],
                                    start=(h == 0), stop=(h == 3))
                            ost = sa.tile([128, 512], F32, name="ost", tag="ost", bufs=3)
                            nc.vector.tensor_copy(ost[:], ps_o[:])
                            nc.sync.dma_start(
                                out[tq0 + 128 * tqc:tq0 + 128 * (tqc + 1),
                                    512 * n:512 * (n + 1)], ost[:])


def _rope_tables():
    inv = 1.0 / (ROPE_BASE ** (np.arange(0, R, 2, dtype=np.float32) / R))
    freqs = np.arange(T, dtype=np.float32)[:, None] * inv[None, :]       # [T, 32]
    emb = np.concatenate([freqs, freqs], axis=-1)                         # [T, 64]
    cosT = np.ascontiguousarray(np.cos(emb).T.astype(np.float32))         # [64, T]
    sinT = np.ascontiguousarray(np.sin(emb).T.astype(np.float32))
    cosd = np.concatenate([cosT, cosT], axis=0)                           # [128, T]
    sin_sgn = np.concatenate([-sinT[0:32], sinT[32:64]], axis=0)          # [64, T]
    sind = np.concatenate([sin_sgn, sin_sgn], axis=0)
    return cosd, sind


def kernel(**inputs):
    from concourse.bass_utils import run_bass_kernel_spmd

    x = np.asarray(inputs["x"], dtype=np.float32)
    W_dq = np.asarray(inputs["W_dq"], dtype=np.float32)
    W_uq = np.asarray(inputs["W_uq"], dtype=np.float32)
    W_qr = np.asarray(inputs["W_qr"], dtype=np.float32)
    W_dkv = np.asarray(inputs["W_dkv"], dtype=np.float32)
    W_uk = np.asarray(inputs["W_uk"], dtype=np.float32)
    W_uv = np.asarray(inputs["W_uv"], dtype=np.float32)
    W_kr = np.asarray(inputs["W_kr"], dtype=np.float32)
    W_o = np.asarray(inputs["W_o"], dtype=np.float32)

    if "nc" not in _CACHE:
        _CACHE["nc"] = _build_nc()
    nc = _CACHE["nc"]

    cosd, sind = _rope_tables()
    maskv = (np.arange(TB)[:, None] <= np.arange(TB)[None, :]).astype(np.float32)
    onesv = np.ones((128, 1), dtype=np.float32)

    in_maps = []
    for core in range(8):
        b, hg = core // 4, core % 4
        in_maps.append({
            "xT": np.ascontiguousarray(x[b].T),
            "wdq": W_dq,
            "wdkv": W_dkv,
            "wuq": np.ascontiguousarray(W_uq[:, 512 * hg:512 * (hg + 1)]),
            "wqr": np.ascontiguousarray(W_qr[:, 256 * hg:256 * (hg + 1)]),
            "wuk": np.ascontiguousarray(W_uk[:, 512 * hg:512 * (hg + 1)]),
            "wuv": np.ascontiguousarray(W_uv[:, 512 * hg:512 * (hg + 1)]),
            "wkr": np.ascontiguousarray(W_kr[:, 256 * hg:256 * (hg + 1)]),
            "wo": np.ascontiguousarray(W_o[512 * hg:512 * (hg + 1), :]),
            "cosd": cosd,
            "sind": sind,
            "maskd": maskv,
            "onesd": onesv,
        })

    res = run_bass_kernel_spmd(nc, in_maps, core_ids=list(range(8)))
    outs = [r["out"] for r in res.results]
    out0 = outs[0] + outs[1] + outs[2] + outs[3]
    out1 = outs[4] + outs[5] + outs[6] + outs[7]
    return np.stack([out0, out1]).astype(np.float32)


# revision 13
# speedup vs baseline: 15569.8255x; 4.3757x over previous
"""Multi-Head Latent Attention (MLA) Bass kernel for 8 trn2 NeuronCores.

Sharding: core c handles batch b=c//4 and head group hg=c%4 (4 of 16 heads).
The small compression projections (W_dq/W_dkv) are replicated; the per-head
up-projections and W_o are sliced by head. Host transposes x[b] once, and the
whole device pipeline runs in "transposed" layout (feature dims on SBUF
partitions) so no on-device transposes are needed:

  c_qT  [1536, T] = W_dq.T  @ x.T      (lhsT = W_dq, rhs = xT)
  qT_h  [128, T]  = W_uq_h.T @ c_qT
  scoresT [tk, tq] = (lhsT=kT_chunk, rhs=qT)
  exp (ACT, no max-subtraction: logits are O(5)), row sums via ones-matmul,
  out_avT [dh, tq] = (lhsT=v_chunk natural, rhs=expT)
  out    [tq, C]  = (lhsT=out_avT, rhs=W_o rows)   + causal block skipping

All matmuls run in float32r (fp32 stored, fp22 multiply) at 1 cycle/row.
Host sums the 4 partial outputs per batch.
"""

import numpy as np

T = 2048
C = 2048
QC = 1536
KV = 512
NH = 16
DH = 128
R = 64
TB = 512           # T block / q-group width
NTB = T // TB      # 4
SCALE = 1.0 / float(np.sqrt(DH + R))
ROPE_BASE = 10000.0

_CACHE = {}


def _build_nc(repeat=1):
    import concourse.bacc as bacc
    import concourse.mybir as mybir
    import concourse.tile as tile

    F32R = mybir.dt.float32r
    F32 = mybir.dt.float32
    AF = mybir.ActivationFunctionType

    nc = bacc.Bacc("TRN2", target_bir_lowering=False, debug=False)

    xT = nc.dram_tensor("xT", [C, T], F32R, kind="ExternalInput")
    wdq = nc.dram_tensor("wdq", [C, QC], F32R, kind="ExternalInput")
    wdkv = nc.dram_tensor("wdkv", [C, KV], F32R, kind="ExternalInput")
    wuq = nc.dram_tensor("wuq", [QC, 512], F32R, kind="ExternalInput")
    wqr = nc.dram_tensor("wqr", [QC, 256], F32R, kind="ExternalInput")
    wuk = nc.dram_tensor("wuk", [KV, 512], F32R, kind="ExternalInput")
    wuv = nc.dram_tensor("wuv", [KV, 512], F32R, kind="ExternalInput")
    wkr = nc.dram_tensor("wkr", [KV, 256], F32R, kind="ExternalInput")
    wo = nc.dram_tensor("wo", [512, C], F32R, kind="ExternalInput")
    cosd = nc.dram_tensor("cosd", [128, T], F32, kind="ExternalInput")
    sind = nc.dram_tensor("sind", [128, T], F32, kind="ExternalInput")
    maskd = nc.dram_tensor("maskd", [TB, TB], F32, kind="ExternalInput")
    onesd = nc.dram_tensor("onesd", [128, 1], F32R, kind="ExternalInput")
    out = nc.dram_tensor("out", [T, C], F32, kind="ExternalOutput")

    with tile.TileContext(nc) as tc:
        with tc.tile_pool(name="dram", bufs=1, space="DRAM") as dpool:
            for _rep in range(repeat):
                _emit_body(nc, tc, dpool, mybir,
                           xT, wdq, wdkv, wuq, wqr, wuk, wuv, wkr, wo,
                           cosd, sind, maskd, onesd, out)

    nc.compile()
    return nc


def _emit_body(nc, tc, dpool, mybir,
               xT, wdq, wdkv, wuq, wqr, wuk, wuv, wkr, wo,
               cosd, sind, maskd, onesd, out):
    F32R = mybir.dt.float32r
    F32 = mybir.dt.float32
    AF = mybir.ActivationFunctionType
    import concourse.tile as tile  # noqa: F401

    if True:
        if True:
            qcT = dpool.tile([512, T], F32R, name="qcT", tag="qcT")
            qrT = dpool.tile([256, T], F32R, name="qrT", tag="qrT")
            kcT = dpool.tile([512, T], F32R, name="kcT", tag="kcT")
            krT = dpool.tile([256, T], F32R, name="krT", tag="krT")
            vS = dpool.tile([T, 512], F32R, name="vS", tag="vS")

            # ---------------- Phase P: projections ----------------
            with (
                tc.tile_pool(name="pp_sb", bufs=1) as sp,
                tc.tile_pool(name="pp_ps", bufs=1, space="PSUM") as pp,
            ):
                # Allocated up front, but the DMAs are emitted after block 0's
                # c_q matmul group so they don't delay the first matmul.
                wuq_sb = sp.tile([128, 12, 512], F32R, name="wuq_sb")
                wqr_sb = sp.tile([128, 12, 256], F32R, name="wqr_sb")
                wuk_sb = sp.tile([128, 4, 512], F32R, name="wuk_sb")
                wuv_sb = sp.tile([128, 4, 512], F32R, name="wuv_sb")
                wkr_sb = sp.tile([128, 4, 256], F32R, name="wkr_sb")

                def load_resident_weights():
                    nc.scalar.dma_start(wuq_sb[:], wuq.rearrange("(k p) n -> p k n", p=128))
                    nc.scalar.dma_start(wqr_sb[:], wqr.rearrange("(k p) n -> p k n", p=128))
                    nc.scalar.dma_start(wuk_sb[:], wuk.rearrange("(k p) n -> p k n", p=128))
                    nc.scalar.dma_start(wuv_sb[:], wuv.rearrange("(k p) n -> p k n", p=128))
                    nc.scalar.dma_start(wkr_sb[:], wkr.rearrange("(k p) n -> p k n", p=128))

                def rope_store(ps_t, dst, p, tc0, cos_sb, sin_sb):
                    # ps_t [128, TB]: rows [64 head 2p | 64 head 2p+1] rope dims
                    t1 = sp.tile([128, TB], F32, name="rp1", tag="rp1", bufs=2)
                    nc.gpsimd.tensor_mul(t1[:], ps_t[:], cos_sb[:])
                    sh = sp.tile([128, TB], F32, name="rp2", tag="rp2", bufs=2)
                    nc.vector.tensor_copy(sh[0:32, :], ps_t[32:64, :])
                    nc.vector.tensor_copy(sh[32:64, :], ps_t[0:32, :])
                    nc.vector.tensor_copy(sh[64:96, :], ps_t[96:128, :])
                    nc.vector.tensor_copy(sh[96:128, :], ps_t[64:96, :])
                    nc.gpsimd.tensor_mul(sh[:], sh[:], sin_sb[:])
                    ro = sp.tile([128, TB], F32R, name="ro", tag="ro", bufs=2)
                    nc.vector.tensor_add(ro[:], t1[:], sh[:])
                    nc.scalar.dma_start(dst[128 * p:128 * (p + 1), tc0:tc0 + TB], ro[:])

                for t in range(NTB):
                    tc0 = TB * t
                    wst0 = sp.tile([128, 16, 128], F32R, name="wst", tag="wst", bufs=3)
                    nc.sync.dma_start(
                        wst0[:], wdq[:, 0:128].rearrange("(k p) n -> p k n", p=128))
                    xblk = sp.tile([128, 16, TB], F32R, name="xblk", tag="xblk", bufs=2)
                    for k in range(16):
                        eng = nc.scalar if k % 2 == 0 else nc.sync
                        eng.dma_start(
                            xblk[:, k, :], xT[128 * k:128 * (k + 1), tc0:tc0 + TB])
                    cos_sb = sp.tile([128, TB], F32, name="cos_sb", tag="cos_sb", bufs=2)
                    nc.scalar.dma_start(cos_sb[:], cosd[:, tc0:tc0 + TB])
                    sin_sb = sp.tile([128, TB], F32, name="sin_sb", tag="sin_sb", bufs=2)
                    nc.scalar.dma_start(sin_sb[:], sind[:, tc0:tc0 + TB])
                    # c_qT block [1536, TB]
                    cq_blk = sp.tile([128, 12, TB], F32R, name="cq_blk", tag="cq_blk", bufs=1)
                    for m in range(12):
                        if m == 0:
                            wst = wst0
                        else:
                            wst = sp.tile([128, 16, 128], F32R, name="wst", tag="wst", bufs=3)
                            nc.sync.dma_start(
                                wst[:], wdq[:, 128 * m:128 * (m + 1)].rearrange(
                                    "(k p) n -> p k n", p=128))
                        ps_t = pp.tile([128, TB], F32, name="ps_p", tag="ps_p", bufs=4)
                        for k in range(16):
                            nc.tensor.matmul(ps_t[:], wst[:, k, :], xblk[:, k, :],
                                             start=(k == 0), stop=(k == 15))
                        nc.vector.tensor_copy(cq_blk[:, m, :], ps_t[:])
                    # c_kvT block [512, TB]
                    ckv_blk = sp.tile([128, 4, TB], F32R, name="ckv_blk", tag="ckv_blk", bufs=1)
                    for m in range(4):
                        wst = sp.tile([128, 16, 128], F32R, name="wst", tag="wst", bufs=3)
                        nc.sync.dma_start(
                            wst[:], wdkv[:, 128 * m:128 * (m + 1)].rearrange(
                                "(k p) n -> p k n", p=128))
                        ps_t = pp.tile([128, TB], F32, name="ps_p", tag="ps_p", bufs=4)
                        for k in range(16):
                            nc.tensor.matmul(ps_t[:], wst[:, k, :], xblk[:, k, :],
                                             start=(k == 0), stop=(k == 15))
                        nc.vector.tensor_copy(ckv_blk[:, m, :], ps_t[:])
                    if t == 0:
                        load_resident_weights()
                    # q content per head
                    for h in range(4):
                        ps_t = pp.tile([128, TB], F32, name="ps_p", tag="ps_p", bufs=4)
                        for k in range(12):
                            nc.tensor.matmul(
                                ps_t[:], wuq_sb[:, k, 128 * h:128 * (h + 1)],
                                cq_blk[:, k, :], start=(k == 0), stop=(k == 11))
                        st = sp.tile([128, TB], F32R, name="stq", tag="stage", bufs=3)
                        nc.vector.tensor_copy(st[:], ps_t[:])
                        nc.scalar.dma_start(qcT[128 * h:128 * (h + 1), tc0:tc0 + TB], st[:])
                    # q rope per head-pair
                    for p in range(2):
                        ps_t = pp.tile([128, TB], F32, name="ps_p", tag="ps_p", bufs=4)
                        for k in range(12):
                            nc.tensor.matmul(
                                ps_t[:], wqr_sb[:, k, 128 * p:128 * (p + 1)],
                                cq_blk[:, k, :], start=(k == 0), stop=(k == 11))
                        rope_store(ps_t, qrT, p, tc0, cos_sb, sin_sb)
                    # k content per head
                    for h in range(4):
                        ps_t = pp.tile([128, TB], F32, name="ps_p", tag="ps_p", bufs=4)
                        for k in range(4):
                            nc.tensor.matmul(
                                ps_t[:], wuk_sb[:, k, 128 * h:128 * (h + 1)],
                                ckv_blk[:, k, :], start=(k == 0), stop=(k == 3))
                        st = sp.tile([128, TB], F32R, name="stk", tag="stage", bufs=3)
                        nc.vector.tensor_copy(st[:], ps_t[:])
                        nc.scalar.dma_start(kcT[128 * h:128 * (h + 1), tc0:tc0 + TB], st[:])
                    # k rope per head-pair
                    for p in range(2):
                        ps_t = pp.tile([128, TB], F32, name="ps_p", tag="ps_p", bufs=4)
                        for k in range(4):
                            nc.tensor.matmul(
                                ps_t[:], wkr_sb[:, k, 128 * p:128 * (p + 1)],
                                ckv_blk[:, k, :], start=(k == 0), stop=(k == 3))
                        rope_store(ps_t, krT, p, tc0, cos_sb, sin_sb)
                    # v natural [tk, 4*dh]
                    for tkc in range(4):
                        ps_t = pp.tile([128, TB], F32, name="ps_p", tag="ps_p", bufs=4)
                        for k in range(4):
                            nc.tensor.matmul(
                                ps_t[:], ckv_blk[:, k, 128 * tkc:128 * (tkc + 1)],
                                wuv_sb[:, k, :], start=(k == 0), stop=(k == 3))
                        st = sp.tile([128, TB], F32R, name="stv", tag="stage", bufs=3)
                        nc.vector.tensor_copy(st[:], ps_t[:])
                        nc.scalar.dma_start(
                            vS[tc0 + 128 * tkc:tc0 + 128 * (tkc + 1), :], st[:])

            # ---------------- Phase A: attention + W_o ----------------
            with (
                tc.tile_pool(name="pa_sb", bufs=1) as sa,
                tc.tile_pool(name="pa_ps", bufs=1, space="PSUM") as pa,
            ):
                wo_sb = sa.tile([128, 4, C], F32R, name="wo_sb")
                nc.sync.dma_start(wo_sb[:], wo.rearrange("(h p) n -> p h n", p=128))
                mask_sb = sa.tile([128, 4, TB], F32, name="mask_sb")
                nc.sync.dma_start(mask_sb[:], maskd.rearrange("(j p) n -> p j n", p=128))
                ones_sb = sa.tile([128, 1], F32R, name="ones_sb")
                nc.sync.dma_start(ones_sb[:], onesd[:])

                for g in range(4):
                    nch = 4 * (g + 1)
                    tq0 = TB * g
                    avn = sa.tile([128, 4 * TB], F32R, name="avn", tag="avn", bufs=2)
                    for h in range(4):
                        pr = 128 * (h // 2) + 64 * (h % 2)
                        qc_t = sa.tile([128, TB], F32R, name="qc_t", tag="qc_t", bufs=2)
                        nc.sync.dma_start(qc_t[:], qcT[128 * h:128 * (h + 1), tq0:tq0 + TB])
                        qr_t = sa.tile([64, TB], F32R, name="qr_t", tag="qr_t", bufs=2)
                        nc.sync.dma_start(qr_t[:], qrT[pr:pr + 64, tq0:tq0 + TB])
                        kc_t = sa.tile([128, T], F32R, name="kc_t", tag="kc_t", bufs=2)
                        nc.sync.dma_start(kc_t[:, 0:128 * nch],
                                          kcT[128 * h:128 * (h + 1), 0:128 * nch])
                        kr_t = sa.tile([64, T], F32R, name="kr_t", tag="kr_t", bufs=2)
                        nc.sync.dma_start(kr_t[:, 0:128 * nch], krT[pr:pr + 64, 0:128 * nch])
                        v_t = sa.tile([128, 16, 128], F32R, name="v_t", tag="v_t", bufs=2)
                        nc.sync.dma_start(
                            v_t[:, 0:nch, :],
                            vS[0:128 * nch, 128 * h:128 * (h + 1)].rearrange(
                                "(c p) d -> p c d", p=128))

                        ps_av = pa.tile([128, TB], F32, name="ps_av", tag="ps_av", bufs=2)
                        ps_sum = pa.tile([1, TB], F32, name="ps_sum", tag="ps_sum", bufs=2)

                        def qk(c):
                            ps_s = pa.tile([128, TB], F32, name="ps_s", tag="ps_s", bufs=2)
                            nc.tensor.matmul(ps_s[:], kc_t[:, 128 * c:128 * (c + 1)],
                                             qc_t[:], start=True, stop=False)
                            nc.tensor.matmul(ps_s[:], kr_t[:, 128 * c:128 * (c + 1)],
                                             qr_t[:], start=False, stop=True)
                            return ps_s

                        cur = qk(0)
                        for c in range(nch):
                            nxt = qk(c + 1) if c + 1 < nch else None
                            ex = sa.tile([128, TB], F32R, name="ex", tag="ex", bufs=3)
                            if c >= 4 * g:
                                er = sa.tile([128, TB], F32, name="er", tag="er", bufs=2)
                                nc.scalar.activation(er[:], cur[:], AF.Exp, scale=SCALE)
                                nc.vector.tensor_mul(ex[:], er[:], mask_sb[:, c - 4 * g, :])
                            else:
                                nc.scalar.activation(ex[:], cur[:], AF.Exp, scale=SCALE)
                            nc.tensor.matmul(ps_av[:], v_t[:, c, :], ex[:],
                                             start=(c == 0), stop=(c == nch - 1))
                            nc.tensor.matmul(ps_sum[:], ones_sb[:], ex[:],
                                             start=(c == 0), stop=(c == nch - 1))
                            cur = nxt
                        recip = sa.tile([1, TB], F32, name="recip", tag="recip", bufs=2)
                        nc.vector.reciprocal(recip[:], ps_sum[:])
                        rb = sa.tile([1, TB], F32, name="rb", tag="rb", bufs=2, space="DRAM")
                        nc.sync.dma_start(rb[:], recip[:])
                        bc = sa.tile([128, TB], F32, name="bc", tag="bc", bufs=2)
                        nc.sync.dma_start(bc[:], rb[0:1, :].to_broadcast([128, TB]))
                        nc.vector.tensor_mul(avn[:, TB * h:TB * (h + 1)], ps_av[:], bc[:])
                    # W_o partial for this q group
                    for tqc in range(4):
                        for n in range(4):
                            ps_o = pa.tile([128, 512], F32, name="ps_o", tag="ps_o", bufs=2)
                            for h in range(4):
                                nc.tensor.matmul(
                                    ps_o[:],
                                    avn[:, TB * h + 128 * tqc:TB * h + 128 * (tqc + 1)],
                                    wo_sb[:, h, 512 * n:512 * (n + 1)],
                                    start=(h == 0), stop=(h == 3))
                            ost = sa.tile([128, 512], F32, name="ost", tag="ost", bufs=3)
                            nc.vector.tensor_copy(ost[:], ps_o[:])
                            nc.sync.dma_start(
                                out[tq0 + 128 * tqc:tq0 + 128 * (tqc + 1),
                                    512 * n:512 * (n + 1)], ost[:])


def _rope_tables():
    inv = 1.0 / (ROPE_BASE ** (np.arange(0, R, 2, dtype=np.float32) / R))
    freqs = np.arange(T, dtype=np.float32)[:, None] * inv[None, :]       # [T, 32]
    emb = np.concatenate([freqs, freqs], axis=-1)                         # [T, 64]
    cosT = np.ascontiguousarray(np.cos(emb).T.astype(np.float32))         # [64, T]
    sinT = np.ascontiguousarray(np.sin(emb).T.astype(np.float32))
    cosd = np.concatenate([cosT, cosT], axis=0)                           # [128, T]
    sin_sgn = np.concatenate([-sinT[0:32], sinT[32:64]], axis=0)          # [64, T]
    sind = np.concatenate([sin_sgn, sin_sgn], axis=0)
    return cosd, sind


def kernel(**inputs):
    from concourse.bass_utils import run_bass_kernel_spmd

    x = np.asarray(inputs["x"], dtype=np.float32)
    W_dq = np.asarray(inputs["W_dq"], dtype=np.float32)
    W_uq = np.asarray(inputs["W_uq"], dtype=np.float32)
    W_qr = np.asarray(inputs["W_qr"], dtype=np.float32)
    W_dkv = np.asarray(inputs["W_dkv"], dtype=np.float32)
    W_uk = np.asarray(inputs["W_uk"], dtype=np.float32)
    W_uv = np.asarray(inputs["W_uv"], dtype=np.float32)
    W_kr = np.asarray(inputs["W_kr"], dtype=np.float32)
    W_o = np.asarray(inputs["W_o"], dtype=np.float32)

    if "nc" not in _CACHE:
        _CACHE["nc"] = _build_nc()
    nc = _CACHE["nc"]

    cosd, sind = _rope_tables()
    maskv = (np.arange(TB)[:, None] <= np.arange(TB)[None, :]).astype(np.float32)
    onesv = np.ones((128, 1), dtype=np.float32)

    in_maps = []
    for core in range(8):
        b, hg = core // 4, core % 4
        in_maps.append({
            "xT": np.ascontiguousarray(x[b].T),
            "wdq": W_dq,
            "wdkv": W_dkv,
            "wuq": np.ascontiguousarray(W_uq[:, 512 * hg:512 * (hg + 1)]),
            "wqr": np.ascontiguousarray(W_qr[:, 256 * hg:256 * (hg + 1)]),
            "wuk": np.ascontiguousarray(W_uk[:, 512 * hg:512 * (hg + 1)]),
            "wuv": np.ascontiguousarray(W_uv[:, 512 * hg:512 * (hg + 1)]),
            "wkr": np.ascontiguousarray(W_kr[:, 256 * hg:256 * (hg + 1)]),
            "wo": np.ascontiguousarray(W_o[512 * hg:512 * (hg + 1), :]),
            "cosd": cosd,
            "sind": sind,
            "maskd": maskv,
            "onesd": onesv,
        })

    res = run_bass_kernel_spmd(nc, in_maps, core_ids=list(range(8)))
    outs = [r["out"] for r in res.results]
    out0 = outs[0] + outs[1] + outs[2] + outs[3]
    out1 = outs[4] + outs[5] + outs[6] + outs[7]
    return np.stack([out0, out1]).astype(np.float32)
